# revision 1
# baseline (speedup 1.0000x reference)
"""Dense transformer block (LN1 -> causal MHA -> proj -> residual -> LN2 ->
FC1 -> gelu(tanh) -> FC2 -> residual) on 8 Trainium2 NeuronCores.

Sharding: two SPMD launches, no on-device collectives.
  Launch 1 (tensor-parallel over batch x head-group): core c = (batch c//4,
    heads 4*(c%4)..4*(c%4)+3). LayerNorm1 is folded into the QKV matmul via
    host-precomputed W*g weights plus augmented contraction rows carrying the
    per-row mean and std, so the kernel works directly on the transposed raw
    input x^T. Attention runs in transposed layout: scores^T = K @ Q^T (softmax
    reduction lands on the free axis of the PV matmul via a ones-column
    appended to V, which accumulates the denominator for free). Each core
    returns its partial w_proj output; the host sums the 4 partials per batch
    (the unshard step for the row-sharded w_proj) and adds the residual.
  Launch 2 (data-parallel over tokens): core c takes 512 of the 4096 rows of
    h' and computes LN2 + FC1 + gelu + FC2 in the same transposed layout;
    the host adds the residual.

All matmul inputs are bf16; every accumulation/statistic is fp32 on-chip.
"""

import sys

if "/opt/trn_rl_repo" not in sys.path:
    sys.path.insert(0, "/opt/trn_rl_repo")

import numpy as np
import ml_dtypes

import concourse.bass as bass
import concourse.tile as tile
from concourse import mybir
import bass_rust
from concourse.bass_utils import run_bass_kernel_spmd

B, S, D, H, DH, DFF = 2, 2048, 1024, 16, 64, 4096
NCORES = 8
HG = 4          # heads per core
QC = 512        # query chunk
KB = 128        # key block
NQ = S // QC    # 4 query chunks
NRT = S // 128  # 16 row tiles
EPS = 1e-5
NEG = -30000.0
ROWS2 = (B * S) // NCORES  # 512 rows per core in launch 2

bf16 = mybir.dt.bfloat16
f32 = mybir.dt.float32
nbf = ml_dtypes.bfloat16

AF = mybir.ActivationFunctionType
ALU = mybir.AluOpType


def _finish(nc):
    bass_rust.move_matmul_waits_to_ldweights(nc.m)
    bass_rust.generate_event_semaphores(nc)
    return nc


# --------------------------------------------------------------------------
# Launch 1: LN1 + QKV + causal attention + partial proj
# --------------------------------------------------------------------------
def build_l1(rep=1):
    nc = bass.Bass()
    d_xT = nc.declare_dram_parameter("xT", [D, S], bf16, isOutput=False)
    d_wqk = nc.declare_dram_parameter("wqk", [D, 512], bf16, isOutput=False)
    d_wqka = nc.declare_dram_parameter("wqka", [1, 1024], bf16, isOutput=False)
    d_wv = nc.declare_dram_parameter("wv", [D, 256], bf16, isOutput=False)
    d_uv = nc.declare_dram_parameter("uv", [1, 256], bf16, isOutput=False)
    d_wp = nc.declare_dram_parameter("wp", [256, 1024], bf16, isOutput=False)
    d_masks = nc.declare_dram_parameter("masks", [4, KB, QC], bf16, isOutput=False)
    d_out = nc.declare_dram_parameter("out", [S, D], f32, isOutput=True)

    with tile.TileContext(nc) as tc:
        with (
            tc.tile_pool(name="const", bufs=1) as const,
            tc.tile_pool(name="persist", bufs=1) as persist,
            tc.tile_pool(name="exps", bufs=6) as exps,
            tc.tile_pool(name="evict", bufs=3) as evict,
            tc.tile_pool(name="pbig", bufs=3, space="PSUM") as pbig,
            tc.tile_pool(name="pacc", bufs=1, space="PSUM") as pacc,
            tc.tile_pool(name="py", bufs=2, space="PSUM") as py,
            tc.tile_pool(name="ptiny", bufs=1, space="PSUM") as ptiny,
        ):
            # ---- constants / persistent tiles ----
            t_wqk = const.tile([128, 8, 512], bf16)
            t_wv = const.tile([128, 8, 256], bf16)
            t_wp = const.tile([64, 4, 1024], bf16)
            t_wqka = const.tile([1, 1024], bf16)
            t_uv = const.tile([1, 256], bf16)
            t_masks = const.tile([128, 4, QC], bf16)
            t_ones = const.tile([128, 128], bf16)
            t_eps = const.tile([1, 1], f32)
            nc.sync.dma_start(t_wqka[:], d_wqka[:])
            nc.sync.dma_start(t_uv[:], d_uv[:])
            nc.sync.dma_start(t_masks[:], d_masks[:].rearrange("j p n -> p j n"))
            for ci in range(8):
                nc.sync.dma_start(t_wqk[:, ci, :], d_wqk[ci * 128:(ci + 1) * 128, :])
                nc.sync.dma_start(t_wv[:, ci, :], d_wv[ci * 128:(ci + 1) * 128, :])
            for hh in range(4):
                nc.sync.dma_start(t_wp[:, hh, :], d_wp[hh * 64:(hh + 1) * 64, :])
            nc.vector.memset(t_ones[:], 1.0)
            nc.vector.memset(t_eps[:], EPS)

            t_xT = persist.tile([128, 8, S], bf16)
            for ci in range(8):
                nc.sync.dma_start(t_xT[:, ci, :], d_xT[ci * 128:(ci + 1) * 128, :])

            # Q01 / K01 / Q23 / K23 transposed pair tiles [128, S]
            t_qk = [persist.tile([128, S], bf16, tag=f"qk{i}", name=f"qk{i}")
                    for i in range(4)]
            # V natural, per row-tile: [128, head, 65] (col 64 = ones)
            t_vau = persist.tile([128, NRT, HG, 65], bf16)
            # Y^T per head [64, S]
            t_y = [persist.tile([64, S], bf16, tag=f"y{h}", name=f"y{h}")
                   for h in range(HG)]
            # per-row LN rows
            t_negmu = persist.tile([1, S], bf16)
            t_std = persist.tile([1, S], bf16)
            t_rstd = persist.tile([1, S], bf16)      # 1/std

            for _r in range(rep):
              # ---- LN stats for all row chunks up front: the small-op
              # chains (psum -> ACT -> DVE) resolve while PE streams the
              # heavy phases, instead of stalling each chunk's QKV ----
              for qi in range(NQ):
                rsl = bass.ts(qi, QC)
                p_sx = ptiny.tile([1, QC], f32, tag="sx")
                p_sx2 = ptiny.tile([1, QC], f32, tag="sx2")
                for ci in range(8):
                    nc.tensor.matmul(p_sx[:], t_ones[:, 0:1],
                                     t_xT[:, ci, rsl],
                                     start=(ci == 0), stop=(ci == 7),
                                     skip_group_check=True)
                for ci in range(8):
                    sq = evict.tile([128, QC], bf16, tag="sq")
                    nc.vector.tensor_mul(out=sq[:], in0=t_xT[:, ci, rsl],
                                         in1=t_xT[:, ci, rsl])
                    nc.tensor.matmul(p_sx2[:], t_ones[:, 0:1],
                                     sq[:],
                                     start=(ci == 0), stop=(ci == 7),
                                     skip_group_check=True)
                # mu, var, std, rstd rows
                mu_f = evict.tile([1, QC], f32, tag="mu")
                m2_f = evict.tile([1, QC], f32, tag="m2")
                nc.scalar.activation(mu_f[:], p_sx[:], AF.Copy, scale=1.0 / D)
                nc.scalar.activation(m2_f[:], p_sx2[:], AF.Copy, scale=1.0 / D)
                nc.scalar.activation(t_negmu[0:1, rsl], p_sx[:], AF.Copy, scale=-1.0 / D)
                var_f = evict.tile([1, QC], f32, tag="var")
                nc.vector.tensor_mul(out=mu_f[:], in0=mu_f[:], in1=mu_f[:])
                nc.vector.tensor_tensor(out=var_f[:], in0=m2_f[:], in1=mu_f[:],
                                        op=ALU.subtract)
                std_f = evict.tile([1, QC], f32, tag="std")
                nc.scalar.activation(std_f[:], var_f[:], AF.Sqrt, bias=t_eps[:])
                nc.scalar.activation(t_std[0:1, rsl], std_f[:], AF.Copy)
                with nc.allow_low_precision(reason="rstd feeds bf16 matmul"):
                    nc.vector.reciprocal(out=t_rstd[0:1, rsl], in_=std_f[:])

              for qi in range(NQ):
                rsl = bass.ts(qi, QC)
                # ================= phase A: QKV^T + V =================
                # broadcast rstd over 128 partitions
                p_bc = pacc.tile([128, QC], f32, tag="acc")
                nc.tensor.matmul(p_bc[:], t_ones[0:1, :], t_rstd[0:1, rsl],
                                 start=True, stop=True)
                rstd_b = evict.tile([128, QC], f32, tag="rstdb")
                nc.vector.tensor_copy(rstd_b[:], p_bc[:])

                # QKV^T for Q/K: 4 column tiles
                for ct in range(4):
                    p_qk = pbig.tile([128, QC], f32, tag="big")
                    csl = bass.ts(ct, 128)
                    for ci in range(8):
                        nc.tensor.matmul(p_qk[:], t_wqk[:, ci, csl],
                                         t_xT[:, ci, rsl],
                                         start=(ci == 0), stop=False)
                    nc.tensor.matmul(p_qk[:], t_wqka[0:1, csl], t_negmu[0:1, rsl],
                                     start=False, stop=False, skip_group_check=True)
                    nc.tensor.matmul(p_qk[:],
                                     t_wqka[0:1, bass.ds(512 + ct * 128, 128)],
                                     t_std[0:1, rsl],
                                     start=False, stop=True, skip_group_check=True)
                    nc.vector.tensor_mul(out=t_qk[ct][:, rsl], in0=p_qk[:],
                                         in1=rstd_b[:])

                # V natural for the 4 row tiles of this chunk
                for rt in range(qi * 4, qi * 4 + 4):
                    rtsl = bass.ts(rt, 128)
                    p_v = pacc.tile([128, 256], f32, tag="acc")
                    for ci in range(8):
                        nc.tensor.matmul(p_v[:], t_xT[:, ci, rtsl],
                                         t_wv[:, ci, :],
                                         start=(ci == 0), stop=False)
                    nc.tensor.matmul(p_v[:], t_negmu[0:1, rtsl], t_uv[:],
                                     start=False, stop=True, skip_group_check=True)
                    # rstd as a natural [128,1] column via K=1 matmul
                    p_t = ptiny.tile([128, 1], f32, tag="sx")
                    nc.tensor.matmul(p_t[:], t_rstd[0:1, rtsl], t_ones[0:1, 0:1],
                                     start=True, stop=True)
                    rstd_n = evict.tile([128, 1], f32, tag="rstdn")
                    nc.vector.tensor_copy(rstd_n[:], p_t[:])
                    nc.vector.tensor_scalar_mul(
                        out=t_vau[:, rt, :, 0:64],
                        in0=p_v[:].rearrange("p (h n) -> p h n", h=HG),
                        scalar1=rstd_n[:])
                    nc.vector.memset(t_vau[:, rt, :, 64:65], 1.0)

                # ================= phase B: attention =================
                for pr in range(2):
                    qt = t_qk[2 * pr]
                    kt = t_qk[2 * pr + 1]
                    nkb = 4 * (qi + 1)
                    hs = [2 * pr, 2 * pr + 1]
                    p_ys = {}
                    for h in hs:
                        p_ys[h] = py.tile([65, QC], f32, tag="y", name=f"py{h}")
                    for kb in range(nkb):
                        diag = kb >= 4 * qi
                        jj = kb - 4 * qi
                        # columns q < 128*jj of a diagonal block are fully
                        # masked -> skip them in scores/exp/mask/PV
                        qo = 128 * jj if diag else 0
                        qn = QC - qo
                        es = {}
                        for h in hs:
                            off = 64 * (h % 2)
                            p_s = pbig.tile([128, QC], f32, tag="big",
                                            name=f"ps{h}")
                            nc.tensor.matmul(
                                p_s[:, qo:QC],
                                kt[off:off + 64, bass.ts(kb, KB)],
                                qt[off:off + 64, bass.ds(qi * QC + qo, qn)],
                                start=True, stop=True,
                                skip_group_check=True)
                            e = exps.tile([128, QC], bf16, tag="e",
                                          name=f"e{h}")
                            nc.scalar.activation(e[:, qo:QC], p_s[:, qo:QC],
                                                 AF.Exp)
                            if diag:
                                nc.vector.tensor_mul(
                                    out=e[:, qo:QC], in0=e[:, qo:QC],
                                    in1=t_masks[:, jj, qo:QC])
                            es[h] = e
                        for h in hs:
                            nc.tensor.matmul(
                                p_ys[h][:, qo:QC],
                                t_vau[:, kb, h, :],
                                es[h][:, qo:QC],
                                start=(kb == 0), stop=(kb == nkb - 1),
                                skip_group_check=True)
                    # normalize: Y[0:64] * broadcast(1/se)
                    for h in hs:
                        p_y = p_ys[h]
                        se = evict.tile([65, QC], bf16, tag="se")
                        with nc.allow_low_precision(reason="softmax denom feeds bf16 matmul"):
                            nc.vector.reciprocal(out=se[64:65, :], in_=p_y[64:65, :])
                        p_n = pacc.tile([64, QC], f32, tag="acc")
                        nc.tensor.matmul(p_n[:], t_ones[64:65, 0:64], se[64:65, :],
                                         start=True, stop=True)
                        bc = evict.tile([64, QC], f32, tag="bc")
                        nc.vector.tensor_copy(bc[:], p_n[:])
                        nc.vector.tensor_mul(out=t_y[h][:, rsl], in0=p_y[0:64, :],
                                             in1=bc[:])

                # ================= phase C: partial proj =================
                for rt in range(qi * 4, qi * 4 + 4):
                    rtsl = bass.ts(rt, 128)
                    o_sb = evict.tile([128, 1024], f32, tag="osb")
                    for nh in range(2):
                        p_o = pbig.tile([128, 512], f32, tag="big")
                        for h in range(HG):
                            nc.tensor.matmul(p_o[:], t_y[h][:, rtsl],
                                             t_wp[:, h, bass.ts(nh, 512)],
                                             start=(h == 0), stop=(h == HG - 1))
                        nc.vector.tensor_copy(o_sb[:, bass.ts(nh, 512)], p_o[:])
                    nc.sync.dma_start(d_out[rtsl, :], o_sb[:])

    return _finish(nc)


# --------------------------------------------------------------------------
# Launch 2: LN2 + FC1 + gelu + FC2 (rows sharded)
# --------------------------------------------------------------------------
def build_l2(rep=1):
    R = ROWS2
    nc = bass.Bass()
    d_hT = nc.declare_dram_parameter("hT", [D, R], bf16, isOutput=False)
    d_wfc = nc.declare_dram_parameter("wfc", [32, 8, 128, 128], bf16, isOutput=False)
    d_ufc = nc.declare_dram_parameter("ufc", [1, DFF], bf16, isOutput=False)
    d_cfc = nc.declare_dram_parameter("cfc", [128, 32], f32, isOutput=False)
    d_w2 = nc.declare_dram_parameter("w2", [8, 32, 128, 128], bf16, isOutput=False)
    d_b2 = nc.declare_dram_parameter("b2", [128, 8], f32, isOutput=False)
    d_out = nc.declare_dram_parameter("out", [D, R], f32, isOutput=True)

    with tile.TileContext(nc) as tc:
        with (
            tc.tile_pool(name="const", bufs=1) as const,
            tc.tile_pool(name="persist", bufs=1) as persist,
            tc.tile_pool(name="wstream", bufs=3) as wstream,
            tc.tile_pool(name="w2stream", bufs=3) as w2stream,
            tc.tile_pool(name="evict", bufs=3) as evict,
            tc.tile_pool(name="pbig", bufs=3, space="PSUM") as pbig,
            tc.tile_pool(name="pacc", bufs=2, space="PSUM") as pacc,
            tc.tile_pool(name="ptiny", bufs=1, space="PSUM") as ptiny,
        ):
            t_ufc = const.tile([1, DFF], bf16)
            t_cfc = const.tile([128, 32], f32)
            t_b2 = const.tile([128, 8], f32)
            t_ones = const.tile([128, 128], bf16)
            t_ones1r = const.tile([1, ROWS2], bf16)
            t_eps = const.tile([1, 1], f32)
            nc.vector.memset(t_ones1r[:], 1.0)
            nc.sync.dma_start(t_ufc[:], d_ufc[:])
            nc.sync.dma_start(t_cfc[:], d_cfc[:])
            nc.sync.dma_start(t_b2[:], d_b2[:])
            nc.vector.memset(t_ones[:], 1.0)
            nc.vector.memset(t_eps[:], EPS)

            t_hT = persist.tile([128, 8, R], bf16)
            for ci in range(8):
                nc.sync.dma_start(t_hT[:, ci, :], d_hT[ci * 128:(ci + 1) * 128, :])

            t_h1 = persist.tile([128, 32, R], bf16)  # gelu outputs, transposed
            t_aug = persist.tile([1, R], bf16)       # -mu row
            t_rstd = persist.tile([1, R], bf16)

            for _r in range(rep):
              # ---- stats ----
              p_sx = ptiny.tile([1, R], f32, tag="sx")
              p_sx2 = ptiny.tile([1, R], f32, tag="sx2")
              for ci in range(8):
                  nc.tensor.matmul(p_sx[:], t_ones[:, 0:1],
                                   t_hT[:, ci, :], start=(ci == 0), stop=(ci == 7),
                                   skip_group_check=True)
              for ci in range(8):
                  sq = evict.tile([128, R], bf16, tag="sq")
                  nc.scalar.activation(sq[:], t_hT[:, ci, :], AF.Square)
                  nc.tensor.matmul(p_sx2[:], t_ones[:, 0:1],
                                   sq[:], start=(ci == 0), stop=(ci == 7),
                                   skip_group_check=True)
              mu_f = evict.tile([1, R], f32, tag="mu")
              m2_f = evict.tile([1, R], f32, tag="m2")
              nc.scalar.activation(mu_f[:], p_sx[:], AF.Copy, scale=1.0 / D)
              nc.scalar.activation(m2_f[:], p_sx2[:], AF.Copy, scale=1.0 / D)
              nc.scalar.activation(t_aug[0:1, :], p_sx[:], AF.Copy, scale=-1.0 / D)
              var_f = evict.tile([1, R], f32, tag="var")
              nc.vector.tensor_mul(out=mu_f[:], in0=mu_f[:], in1=mu_f[:])
              nc.vector.tensor_tensor(out=var_f[:], in0=m2_f[:], in1=mu_f[:],
                                      op=ALU.subtract)
              std_f = evict.tile([1, R], f32, tag="std")
              nc.scalar.activation(std_f[:], var_f[:], AF.Sqrt, bias=t_eps[:])
              with nc.allow_low_precision(reason="rstd feeds bf16 matmul"):
                  nc.vector.reciprocal(out=t_rstd[0:1, :], in_=std_f[:])

              p_bc = pacc.tile([128, R], f32, tag="acc")
              nc.tensor.matmul(p_bc[:], t_ones[0:1, :], t_rstd[0:1, :],
                               start=True, stop=True)
              rstd_b = evict.tile([128, R], f32, tag="rstdb")
              nc.vector.tensor_copy(rstd_b[:], p_bc[:])

              # ---- FC1 + gelu ----
              for ct in range(32):
                  wt = wstream.tile([128, 8, 128], bf16, tag="wfc")
                  nc.sync.dma_start(wt[:], d_wfc[ct].rearrange("c p n -> p c n"))
                  p1 = pbig.tile([128, R], f32, tag="big")
                  for ci in range(8):
                      nc.tensor.matmul(p1[:], wt[:, ci, :], t_hT[:, ci, :],
                                       start=(ci == 0), stop=False)
                  nc.tensor.matmul(p1[:], t_ufc[0:1, bass.ts(ct, 128)], t_aug[0:1, :],
                                   start=False, stop=True, skip_group_check=True)
                  t1 = evict.tile([128, R], f32, tag="t1")
                  nc.vector.tensor_mul(out=t1[:], in0=p1[:], in1=rstd_b[:])
                  nc.scalar.activation(t_h1[:, ct, :], t1[:], AF.Gelu_apprx_tanh,
                                       bias=t_cfc[:, ct:ct + 1])

              # ---- FC2 ----
              for ct2 in range(8):
                  w2t = w2stream.tile([128, 32, 128], bf16, tag="w2")
                  nc.sync.dma_start(w2t[:], d_w2[ct2].rearrange("c p n -> p c n"))
                  p2 = pbig.tile([128, R], f32, tag="big")
                  for ci in range(32):
                      nc.tensor.matmul(p2[:], w2t[:, ci, :], t_h1[:, ci, :],
                                       start=(ci == 0), stop=(ci == 31))
                  o_sb = evict.tile([128, R], f32, tag="osb")
                  nc.vector.tensor_scalar(out=o_sb[:], in0=p2[:],
                                          scalar1=t_b2[:, ct2:ct2 + 1],
                                          scalar2=None, op0=ALU.add)
                  nc.sync.dma_start(d_out[ct2 * 128:(ct2 + 1) * 128, :], o_sb[:])

    return _finish(nc)


# --------------------------------------------------------------------------
# Host glue
# --------------------------------------------------------------------------
_CACHE = {}


def _get_l1():
    if "l1" not in _CACHE:
        _CACHE["l1"] = build_l1()
    return _CACHE["l1"]


def _get_l2():
    if "l2" not in _CACHE:
        _CACHE["l2"] = build_l2()
    return _CACHE["l2"]


def _make_masks():
    k = np.arange(KB)[:, None]
    q = np.arange(QC)[None, :]
    m = np.zeros((4, KB, QC), np.float32)
    for jj in range(4):
        m[jj] = np.where(128 * jj + k <= q, 1.0, 0.0)
    return m.astype(nbf)


def prep_l1_inputs(hidden_states, ln1_g, ln1_b, w_attn, b_attn):
    h = np.asarray(hidden_states, np.float32)
    g1 = np.asarray(ln1_g, np.float32)
    b1 = np.asarray(ln1_b, np.float32)
    wa = np.asarray(w_attn, np.float32)
    ba = np.asarray(b_attn, np.float32)
    wg = wa * g1[:, None]
    const_all = b1 @ wa + ba  # [3D]
    masks = _make_masks()
    xT = [np.ascontiguousarray(h[b].T).astype(nbf) for b in range(B)]
    sc = 1.0 / np.sqrt(DH)

    in_maps = []
    for c in range(NCORES):
        b, g = c // HG, c % HG
        heads = [HG * g + i for i in range(HG)]
        qcols, kcols, vcols = [], [], []
        for hh in heads:
            qcols += list(range(DH * hh, DH * hh + DH))
            kcols += list(range(D + DH * hh, D + DH * hh + DH))
            vcols += list(range(2 * D + DH * hh, 2 * D + DH * hh + DH))
        # wqk col order: Q01 | K01 | Q23 | K23
        cols = (qcols[:128] + kcols[:128] + qcols[128:] + kcols[128:])
        scale = np.array([sc] * 128 + [1.0] * 128 + [sc] * 128 + [1.0] * 128,
                         np.float32)
        wqk = (wg[:, cols] * scale[None, :]).astype(nbf)
        u = wg[:, cols].sum(axis=0) * scale
        cst = const_all[cols] * scale
        wqka = np.concatenate([u, cst])[None, :].astype(nbf)
        wv = wg[:, vcols].astype(nbf)
        uv = wg[:, vcols].sum(axis=0)[None, :].astype(nbf)
        wp = np.asarray(_CACHE["w_proj_rows"][g], nbf)
        in_maps.append({
            "xT": xT[b], "wqk": wqk, "wqka": wqka, "wv": wv, "uv": uv,
            "wp": wp, "masks": masks,
        })
    return in_maps


def prep_l2_inputs(hp_flat, ln2_g, ln2_b, w_fc, b_fc, w_fc2, b_fc2):
    g2 = np.asarray(ln2_g, np.float32)
    b2 = np.asarray(ln2_b, np.float32)
    wfc = np.asarray(w_fc, np.float32)
    bfc = np.asarray(b_fc, np.float32)
    w2 = np.asarray(w_fc2, np.float32)
    b22 = np.asarray(b_fc2, np.float32)

    wfc_g = (wfc * g2[:, None]).astype(nbf)
    wfc_t = np.ascontiguousarray(
        wfc_g.reshape(8, 128, 32, 128).transpose(2, 0, 1, 3))  # [32ct, 8ci, 128, 128]
    ufc = wfc_g.astype(np.float32).sum(axis=0)[None, :].astype(nbf)
    cfc = (b2 @ wfc + bfc).astype(np.float32).reshape(32, 128).T.copy()  # [128, 32]
    w2_t = np.ascontiguousarray(
        w2.astype(nbf).reshape(32, 128, 8, 128).transpose(2, 0, 1, 3))  # [8ct2, 32ci,...]
    b2cols = b22.astype(np.float32).reshape(8, 128).T.copy()

    in_maps = []
    for c in range(NCORES):
        rows = slice(c * ROWS2, (c + 1) * ROWS2)
        hT = np.ascontiguousarray(hp_flat[rows].T).astype(nbf)
        in_maps.append({
            "hT": hT, "wfc": wfc_t, "ufc": ufc, "cfc": cfc,
            "w2": w2_t, "b2": b2cols,
        })
    return in_maps


def combine_l1(hidden_states, parts, b_attn, ln1_b, w_attn, b_proj):
    h = np.asarray(hidden_states, np.float32)
    wa = np.asarray(w_attn, np.float32)
    const_v = (np.asarray(ln1_b, np.float32) @ wa[:, 2 * D:]
               + np.asarray(b_attn, np.float32)[2 * D:])
    y_const = const_v @ np.asarray(_CACHE["w_proj_full"], np.float32) \
        + np.asarray(b_proj, np.float32)
    hp = h.copy()
    for b in range(B):
        acc = np.zeros((S, D), np.float32)
        for g in range(HG):
            acc += parts[b * HG + g]
        hp[b] += acc + y_const[None, :]
    return hp


def kernel(hidden_states, ln1_g, ln1_b, w_attn, b_attn, w_proj, b_proj,
           ln2_g, ln2_b, w_fc, b_fc, w_fc2, b_fc2):
    wpj = np.asarray(w_proj, np.float32)
    _CACHE["w_proj_full"] = wpj
    _CACHE["w_proj_rows"] = [
        np.concatenate([wpj[DH * (HG * g + i):DH * (HG * g + i) + DH, :]
                        for i in range(HG)], axis=0)
        for g in range(HG)
    ]

    nc1 = _get_l1()
    in1 = prep_l1_inputs(hidden_states, ln1_g, ln1_b, w_attn, b_attn)
    res1 = run_bass_kernel_spmd(nc1, in1, list(range(NCORES)))
    parts = [res1.results[c]["out"] for c in range(NCORES)]

    hp = combine_l1(hidden_states, parts, b_attn, ln1_b, w_attn, b_proj)
    hp_flat = hp.reshape(B * S, D)

    nc2 = _get_l2()
    in2 = prep_l2_inputs(hp_flat, ln2_g, ln2_b, w_fc, b_fc, w_fc2, b_fc2)
    res2 = run_bass_kernel_spmd(nc2, in2, list(range(NCORES)))

    out = hp_flat.copy()
    for c in range(NCORES):
        out[c * ROWS2:(c + 1) * ROWS2] += res2.results[c]["out"].T
    return out.reshape(B, S, D).astype(np.float32)



# revision 10
# speedup vs baseline: 1.0120x; 1.0120x over previous
"""Dense transformer block (LN1 -> causal MHA -> proj -> residual -> LN2 ->
FC1 -> gelu(tanh) -> FC2 -> residual) on 8 Trainium2 NeuronCores.

Sharding: two SPMD launches, no on-device collectives.
  Launch 1 (tensor-parallel over batch x head-group): core c = (batch c//4,
    heads 4*(c%4)..4*(c%4)+3). LayerNorm1 is folded into the QKV matmul via
    host-precomputed W*g weights plus augmented contraction rows carrying the
    per-row mean and std, so the kernel works directly on the transposed raw
    input x^T. Attention runs in transposed layout: scores^T = K @ Q^T (softmax
    reduction lands on the free axis of the PV matmul via a ones-column
    appended to V, which accumulates the denominator for free). Each core
    returns its partial w_proj output; the host sums the 4 partials per batch
    (the unshard step for the row-sharded w_proj) and adds the residual.
  Launch 2 (data-parallel over tokens): core c takes 512 of the 4096 rows of
    h' and computes LN2 + FC1 + gelu + FC2 in the same transposed layout;
    the host adds the residual.

All matmul inputs are bf16; every accumulation/statistic is fp32 on-chip.
"""

import sys

if "/opt/trn_rl_repo" not in sys.path:
    sys.path.insert(0, "/opt/trn_rl_repo")

import numpy as np
import ml_dtypes

import concourse.bass as bass
import concourse.tile as tile
from concourse import mybir
import bass_rust
from concourse.bass_utils import run_bass_kernel_spmd

B, S, D, H, DH, DFF = 2, 2048, 1024, 16, 64, 4096
NCORES = 8
HG = 4          # heads per core
QC = 512        # query chunk
KB = 128        # key block
NQ = S // QC    # 4 query chunks
NRT = S // 128  # 16 row tiles
EPS = 1e-5
NEG = -30000.0
ROWS2 = (B * S) // NCORES  # 512 rows per core in launch 2

bf16 = mybir.dt.bfloat16
f32 = mybir.dt.float32
nbf = ml_dtypes.bfloat16

AF = mybir.ActivationFunctionType
ALU = mybir.AluOpType


def _finish(nc):
    bass_rust.move_matmul_waits_to_ldweights(nc.m)
    bass_rust.generate_event_semaphores(nc)
    return nc


# --------------------------------------------------------------------------
# Launch 1: LN1 + QKV + causal attention + partial proj
# --------------------------------------------------------------------------
def build_l1(rep=1):
    nc = bass.Bass()
    d_xT = nc.declare_dram_parameter("xT", [D, S], bf16, isOutput=False)
    d_wqk = nc.declare_dram_parameter("wqk", [D, 512], bf16, isOutput=False)
    d_wqka = nc.declare_dram_parameter("wqka", [1, 1024], bf16, isOutput=False)
    d_wv = nc.declare_dram_parameter("wv", [D, 256], bf16, isOutput=False)
    d_uv = nc.declare_dram_parameter("uv", [1, 256], bf16, isOutput=False)
    d_wp = nc.declare_dram_parameter("wp", [256, 1024], bf16, isOutput=False)
    d_masks = nc.declare_dram_parameter("masks", [4, KB, QC], bf16, isOutput=False)
    d_out = nc.declare_dram_parameter("out", [S, D], f32, isOutput=True)

    with tile.TileContext(nc) as tc:
        with (
            tc.tile_pool(name="const", bufs=1) as const,
            tc.tile_pool(name="persist", bufs=1) as persist,
            tc.tile_pool(name="exps", bufs=6) as exps,
            tc.tile_pool(name="evict", bufs=3) as evict,
            tc.tile_pool(name="pbig", bufs=3, space="PSUM") as pbig,
            tc.tile_pool(name="pacc", bufs=1, space="PSUM") as pacc,
            tc.tile_pool(name="py", bufs=2, space="PSUM") as py,
            tc.tile_pool(name="ptiny", bufs=1, space="PSUM") as ptiny,
        ):
            # ---- constants / persistent tiles ----
            t_wqk = const.tile([128, 8, 512], bf16)
            t_wv = const.tile([128, 8, 256], bf16)
            t_wp = const.tile([64, 4, 1024], bf16)
            t_wqka = const.tile([1, 1024], bf16)
            t_uv = const.tile([1, 256], bf16)
            t_masks = const.tile([128, 4, QC], bf16)
            t_ones = const.tile([128, 128], bf16)
            t_eps = const.tile([1, 1], f32)
            nc.sync.dma_start(t_wqka[:], d_wqka[:])
            nc.sync.dma_start(t_uv[:], d_uv[:])
            nc.sync.dma_start(t_masks[:], d_masks[:].rearrange("j p n -> p j n"))
            for ci in range(8):
                nc.sync.dma_start(t_wqk[:, ci, :], d_wqk[ci * 128:(ci + 1) * 128, :])
                nc.sync.dma_start(t_wv[:, ci, :], d_wv[ci * 128:(ci + 1) * 128, :])
            for hh in range(4):
                nc.sync.dma_start(t_wp[:, hh, :], d_wp[hh * 64:(hh + 1) * 64, :])
            nc.vector.memset(t_ones[:], 1.0)
            nc.vector.memset(t_eps[:], EPS)

            t_xT = persist.tile([128, 8, S], bf16)
            for ci in range(8):
                nc.sync.dma_start(t_xT[:, ci, :], d_xT[ci * 128:(ci + 1) * 128, :])

            # Q01 / K01 / Q23 / K23 transposed pair tiles [128, S]
            t_qk = [persist.tile([128, S], bf16, tag=f"qk{i}", name=f"qk{i}")
                    for i in range(4)]
            # V natural, per row-tile: [128, head, 65] (col 64 = ones)
            t_vau = persist.tile([128, NRT, HG, 65], bf16)
            # Y^T per head [64, S]
            t_y = [persist.tile([64, S], bf16, tag=f"y{h}", name=f"y{h}")
                   for h in range(HG)]
            # per-row LN rows
            t_negmu = persist.tile([1, S], bf16)
            t_std = persist.tile([1, S], bf16)
            t_rstd = persist.tile([1, S], bf16)      # 1/std

            for _r in range(rep):
              # ---- LN stats for all row chunks up front: the small-op
              # chains (psum -> ACT -> DVE) resolve while PE streams the
              # heavy phases, instead of stalling each chunk's QKV ----
              for qi in range(NQ):
                rsl = bass.ts(qi, QC)
                p_sx = ptiny.tile([1, QC], f32, tag="sx")
                p_sx2 = ptiny.tile([1, QC], f32, tag="sx2")
                for ci in range(8):
                    nc.tensor.matmul(p_sx[:], t_ones[:, 0:1],
                                     t_xT[:, ci, rsl],
                                     start=(ci == 0), stop=(ci == 7),
                                     skip_group_check=True)
                for ci in range(8):
                    sq = evict.tile([128, QC], bf16, tag="sq")
                    nc.vector.tensor_mul(out=sq[:], in0=t_xT[:, ci, rsl],
                                         in1=t_xT[:, ci, rsl])
                    nc.tensor.matmul(p_sx2[:], t_ones[:, 0:1],
                                     sq[:],
                                     start=(ci == 0), stop=(ci == 7),
                                     skip_group_check=True)
                # mu, var, std, rstd rows
                mu_f = evict.tile([1, QC], f32, tag="mu")
                m2_f = evict.tile([1, QC], f32, tag="m2")
                nc.scalar.activation(mu_f[:], p_sx[:], AF.Copy, scale=1.0 / D)
                nc.scalar.activation(m2_f[:], p_sx2[:], AF.Copy, scale=1.0 / D)
                nc.scalar.activation(t_negmu[0:1, rsl], p_sx[:], AF.Copy, scale=-1.0 / D)
                var_f = evict.tile([1, QC], f32, tag="var")
                nc.vector.tensor_mul(out=mu_f[:], in0=mu_f[:], in1=mu_f[:])
                nc.vector.tensor_tensor(out=var_f[:], in0=m2_f[:], in1=mu_f[:],
                                        op=ALU.subtract)
                std_f = evict.tile([1, QC], f32, tag="std")
                nc.scalar.activation(std_f[:], var_f[:], AF.Sqrt, bias=t_eps[:])
                nc.scalar.activation(t_std[0:1, rsl], std_f[:], AF.Copy)
                with nc.allow_low_precision(reason="rstd feeds bf16 matmul"):
                    nc.vector.reciprocal(out=t_rstd[0:1, rsl], in_=std_f[:])

              for qi in range(NQ):
                rsl = bass.ts(qi, QC)
                # ================= phase A: QKV^T + V =================
                # broadcast rstd over 128 partitions
                p_bc = pacc.tile([128, QC], f32, tag="acc")
                nc.tensor.matmul(p_bc[:], t_ones[0:1, :], t_rstd[0:1, rsl],
                                 start=True, stop=True)
                rstd_b = evict.tile([128, QC], f32, tag="rstdb")
                nc.vector.tensor_copy(rstd_b[:], p_bc[:])

                # QKV^T for Q/K: 4 column tiles
                for ct in range(4):
                    p_qk = pbig.tile([128, QC], f32, tag="big")
                    csl = bass.ts(ct, 128)
                    for ci in range(8):
                        nc.tensor.matmul(p_qk[:], t_wqk[:, ci, csl],
                                         t_xT[:, ci, rsl],
                                         start=(ci == 0), stop=False)
                    nc.tensor.matmul(p_qk[:], t_wqka[0:1, csl], t_negmu[0:1, rsl],
                                     start=False, stop=False, skip_group_check=True)
                    nc.tensor.matmul(p_qk[:],
                                     t_wqka[0:1, bass.ds(512 + ct * 128, 128)],
                                     t_std[0:1, rsl],
                                     start=False, stop=True, skip_group_check=True)
                    nc.vector.tensor_mul(out=t_qk[ct][:, rsl], in0=p_qk[:],
                                         in1=rstd_b[:])

                # V natural for the 4 row tiles of this chunk
                for rt in range(qi * 4, qi * 4 + 4):
                    rtsl = bass.ts(rt, 128)
                    p_v = pacc.tile([128, 256], f32, tag="acc")
                    for ci in range(8):
                        nc.tensor.matmul(p_v[:], t_xT[:, ci, rtsl],
                                         t_wv[:, ci, :],
                                         start=(ci == 0), stop=False)
                    nc.tensor.matmul(p_v[:], t_negmu[0:1, rtsl], t_uv[:],
                                     start=False, stop=True, skip_group_check=True)
                    # rstd as a natural [128,1] column via K=1 matmul
                    p_t = ptiny.tile([128, 1], f32, tag="sx")
                    nc.tensor.matmul(p_t[:], t_rstd[0:1, rtsl], t_ones[0:1, 0:1],
                                     start=True, stop=True)
                    rstd_n = evict.tile([128, 1], f32, tag="rstdn")
                    nc.vector.tensor_copy(rstd_n[:], p_t[:])
                    nc.vector.tensor_scalar_mul(
                        out=t_vau[:, rt, :, 0:64],
                        in0=p_v[:].rearrange("p (h n) -> p h n", h=HG),
                        scalar1=rstd_n[:])
                    nc.vector.memset(t_vau[:, rt, :, 64:65], 1.0)

                # ================= phase B: attention =================
                for pr in range(2):
                    qt = t_qk[2 * pr]
                    kt = t_qk[2 * pr + 1]
                    nkb = 4 * (qi + 1)
                    hs = [2 * pr, 2 * pr + 1]
                    p_ys = {}
                    for h in hs:
                        p_ys[h] = py.tile([65, QC], f32, tag="y", name=f"py{h}")
                    for kb in range(nkb):
                        diag = kb >= 4 * qi
                        jj = kb - 4 * qi
                        # columns q < 128*jj of a diagonal block are fully
                        # masked -> skip them in scores/exp/mask/PV
                        qo = 128 * jj if diag else 0
                        qn = QC - qo
                        es = {}
                        for h in hs:
                            off = 64 * (h % 2)
                            p_s = pbig.tile([128, QC], f32, tag="big",
                                            name=f"ps{h}")
                            nc.tensor.matmul(
                                p_s[:, qo:QC],
                                kt[off:off + 64, bass.ts(kb, KB)],
                                qt[off:off + 64, bass.ds(qi * QC + qo, qn)],
                                start=True, stop=True,
                                skip_group_check=True)
                            e = exps.tile([128, QC], bf16, tag="e",
                                          name=f"e{h}")
                            nc.scalar.activation(e[:, qo:QC], p_s[:, qo:QC],
                                                 AF.Exp)
                            if diag:
                                nc.vector.tensor_mul(
                                    out=e[:, qo:QC], in0=e[:, qo:QC],
                                    in1=t_masks[:, jj, qo:QC])
                            es[h] = e
                        for h in hs:
                            nc.tensor.matmul(
                                p_ys[h][:, qo:QC],
                                t_vau[:, kb, h, :],
                                es[h][:, qo:QC],
                                start=(kb == 0), stop=(kb == nkb - 1),
                                skip_group_check=True)
                    # normalize: Y[0:64] * broadcast(1/se)
                    for h in hs:
                        p_y = p_ys[h]
                        se = evict.tile([65, QC], bf16, tag="se")
                        with nc.allow_low_precision(reason="softmax denom feeds bf16 matmul"):
                            nc.vector.reciprocal(out=se[64:65, :], in_=p_y[64:65, :])
                        p_n = pacc.tile([64, QC], f32, tag="acc")
                        nc.tensor.matmul(p_n[:], t_ones[64:65, 0:64], se[64:65, :],
                                         start=True, stop=True)
                        bc = evict.tile([64, QC], f32, tag="bc")
                        nc.vector.tensor_copy(bc[:], p_n[:])
                        nc.vector.tensor_mul(out=t_y[h][:, rsl], in0=p_y[0:64, :],
                                             in1=bc[:])

                # ================= phase C: partial proj =================
                for rt in range(qi * 4, qi * 4 + 4):
                    rtsl = bass.ts(rt, 128)
                    o_sb = evict.tile([128, 1024], f32, tag="osb")
                    for nh in range(2):
                        p_o = pbig.tile([128, 512], f32, tag="big")
                        for h in range(HG):
                            nc.tensor.matmul(p_o[:], t_y[h][:, rtsl],
                                             t_wp[:, h, bass.ts(nh, 512)],
                                             start=(h == 0), stop=(h == HG - 1))
                        nc.vector.tensor_copy(o_sb[:, bass.ts(nh, 512)], p_o[:])
                    nc.sync.dma_start(d_out[rtsl, :], o_sb[:])

    return _finish(nc)


# --------------------------------------------------------------------------
# Launch 2: LN2 + FC1 + gelu + FC2 (rows sharded)
# --------------------------------------------------------------------------
def build_l2(rep=1):
    R = ROWS2
    nc = bass.Bass()
    d_hT = nc.declare_dram_parameter("hT", [D, R], bf16, isOutput=False)
    d_wfc = nc.declare_dram_parameter("wfc", [32, 8, 128, 128], bf16, isOutput=False)
    d_cfc = nc.declare_dram_parameter("cfc", [128, 32], f32, isOutput=False)
    d_w2 = nc.declare_dram_parameter("w2", [8, 32, 128, 128], bf16, isOutput=False)
    d_b2 = nc.declare_dram_parameter("b2", [128, 8], f32, isOutput=False)
    d_out = nc.declare_dram_parameter("out", [D, R], f32, isOutput=True)

    with tile.TileContext(nc) as tc:
        with (
            tc.tile_pool(name="const", bufs=1) as const,
            tc.tile_pool(name="persist", bufs=1) as persist,
            tc.tile_pool(name="wstream", bufs=3) as wstream,
            tc.tile_pool(name="w2stream", bufs=3) as w2stream,
            tc.tile_pool(name="evict", bufs=3) as evict,
            tc.tile_pool(name="pbig", bufs=3, space="PSUM") as pbig,
            tc.tile_pool(name="pacc", bufs=1, space="PSUM") as pacc,
            tc.tile_pool(name="ptiny", bufs=1, space="PSUM") as ptiny,
        ):
            t_cfc = const.tile([128, 32], f32)
            t_b2 = const.tile([128, 8], f32)
            t_ones = const.tile([128, 1], bf16)
            t_onesr = const.tile([1, 128], f32)
            t_eps = const.tile([1, 1], f32)
            nc.sync.dma_start(t_cfc[:], d_cfc[:])
            nc.sync.dma_start(t_b2[:], d_b2[:])
            nc.vector.memset(t_ones[:], 1.0)
            nc.vector.memset(t_onesr[:], 1.0)
            nc.vector.memset(t_eps[:], EPS)

            t_hT = persist.tile([128, 8, R], bf16)
            for ci in range(8):
                nc.sync.dma_start(t_hT[:, ci, :], d_hT[ci * 128:(ci + 1) * 128, :])

            t_wfc = persist.tile([128, 32, 8, 128], bf16)
            for ct in range(32):
                nc.sync.dma_start(t_wfc[:, ct, :, :],
                                  d_wfc[ct].rearrange("c p n -> p c n"))

            t_xn = persist.tile([128, 8, R], bf16)   # normalized input
            t_h1 = persist.tile([128, 32, R], bf16)  # gelu outputs, transposed
            t_aug = persist.tile([1, R], f32)        # -mu row
            t_rstd = persist.tile([1, R], f32)

            for _r in range(rep):
              # ---- stats ----
              p_sx = ptiny.tile([1, R], f32, tag="sx")
              p_sx2 = ptiny.tile([1, R], f32, tag="sx2")
              for ci in range(8):
                  nc.tensor.matmul(p_sx[:], t_ones[:, 0:1],
                                   t_hT[:, ci, :], start=(ci == 0), stop=(ci == 7),
                                   skip_group_check=True)
              for ci in range(8):
                  sq = evict.tile([128, R], bf16, tag="sq")
                  nc.scalar.activation(sq[:], t_hT[:, ci, :], AF.Square)
                  nc.tensor.matmul(p_sx2[:], t_ones[:, 0:1],
                                   sq[:], start=(ci == 0), stop=(ci == 7),
                                   skip_group_check=True)
              mu_f = evict.tile([1, R], f32, tag="mu")
              m2_f = evict.tile([1, R], f32, tag="m2")
              nc.scalar.activation(mu_f[:], p_sx[:], AF.Copy, scale=1.0 / D)
              nc.scalar.activation(m2_f[:], p_sx2[:], AF.Copy, scale=1.0 / D)
              nc.scalar.activation(t_aug[0:1, :], p_sx[:], AF.Copy, scale=-1.0 / D)
              var_f = evict.tile([1, R], f32, tag="var")
              nc.vector.tensor_mul(out=mu_f[:], in0=mu_f[:], in1=mu_f[:])
              nc.vector.tensor_tensor(out=var_f[:], in0=m2_f[:], in1=mu_f[:],
                                      op=ALU.subtract)
              std_f = evict.tile([1, R], f32, tag="std")
              nc.scalar.activation(std_f[:], var_f[:], AF.Sqrt, bias=t_eps[:])
              nc.vector.reciprocal(out=t_rstd[0:1, :], in_=std_f[:])

              # ---- broadcast -mu and rstd over partitions (PE rank-1) ----
              p_nm = pacc.tile([128, R], f32, tag="nm")
              p_rs = pacc.tile([128, R], f32, tag="rs")
              nc.tensor.matmul(p_nm[:], t_onesr[:], t_aug[0:1, :],
                               start=True, stop=True)
              nc.tensor.matmul(p_rs[:], t_onesr[:], t_rstd[0:1, :],
                               start=True, stop=True)

              # ---- normalize input: xn = (h - mu) * rstd ----
              for ci in range(8):
                  cen = evict.tile([128, R], f32, tag="cen")
                  nc.vector.tensor_tensor(out=cen[:], in0=t_hT[:, ci, :],
                                          in1=p_nm[:], op=ALU.add)
                  nc.vector.tensor_mul(out=t_xn[:, ci, :], in0=cen[:],
                                       in1=p_rs[:])

              # ---- FC1 + gelu ----
              for ct in range(32):
                  p1 = pbig.tile([128, R], f32, tag="big")
                  for ci in range(8):
                      nc.tensor.matmul(p1[:], t_wfc[:, ct, ci, :], t_xn[:, ci, :],
                                       start=(ci == 0), stop=(ci == 7))
                  nc.scalar.activation(t_h1[:, ct, :], p1[:], AF.Gelu_apprx_tanh,
                                       bias=t_cfc[:, ct:ct + 1])

              # ---- FC2 ----
              for ct2 in range(8):
                  w2t = w2stream.tile([128, 32, 128], bf16, tag="w2")
                  nc.sync.dma_start(w2t[:], d_w2[ct2].rearrange("c p n -> p c n"))
                  p2 = pbig.tile([128, R], f32, tag="big")
                  for ci in range(32):
                      nc.tensor.matmul(p2[:], w2t[:, ci, :], t_h1[:, ci, :],
                                       start=(ci == 0), stop=(ci == 31))
                  o_sb = evict.tile([128, R], f32, tag="osb")
                  nc.vector.tensor_scalar(out=o_sb[:], in0=p2[:],
                                          scalar1=t_b2[:, ct2:ct2 + 1],
                                          scalar2=None, op0=ALU.add)
                  nc.sync.dma_start(d_out[ct2 * 128:(ct2 + 1) * 128, :], o_sb[:])

    return _finish(nc)


# --------------------------------------------------------------------------
# Host glue
# --------------------------------------------------------------------------
_CACHE = {}


def _get_l1():
    if "l1" not in _CACHE:
        _CACHE["l1"] = build_l1()
    return _CACHE["l1"]


def _get_l2():
    if "l2" not in _CACHE:
        _CACHE["l2"] = build_l2()
    return _CACHE["l2"]


def _make_masks():
    k = np.arange(KB)[:, None]
    q = np.arange(QC)[None, :]
    m = np.zeros((4, KB, QC), np.float32)
    for jj in range(4):
        m[jj] = np.where(128 * jj + k <= q, 1.0, 0.0)
    return m.astype(nbf)


def prep_l1_inputs(hidden_states, ln1_g, ln1_b, w_attn, b_attn):
    h = np.asarray(hidden_states, np.float32)
    g1 = np.asarray(ln1_g, np.float32)
    b1 = np.asarray(ln1_b, np.float32)
    wa = np.asarray(w_attn, np.float32)
    ba = np.asarray(b_attn, np.float32)
    wg = wa * g1[:, None]
    const_all = b1 @ wa + ba  # [3D]
    masks = _make_masks()
    xT = [np.ascontiguousarray(h[b].T).astype(nbf) for b in range(B)]
    sc = 1.0 / np.sqrt(DH)

    in_maps = []
    for c in range(NCORES):
        b, g = c // HG, c % HG
        heads = [HG * g + i for i in range(HG)]
        qcols, kcols, vcols = [], [], []
        for hh in heads:
            qcols += list(range(DH * hh, DH * hh + DH))
            kcols += list(range(D + DH * hh, D + DH * hh + DH))
            vcols += list(range(2 * D + DH * hh, 2 * D + DH * hh + DH))
        # wqk col order: Q01 | K01 | Q23 | K23
        cols = (qcols[:128] + kcols[:128] + qcols[128:] + kcols[128:])
        scale = np.array([sc] * 128 + [1.0] * 128 + [sc] * 128 + [1.0] * 128,
                         np.float32)
        wqk = (wg[:, cols] * scale[None, :]).astype(nbf)
        u = wg[:, cols].sum(axis=0) * scale
        cst = const_all[cols] * scale
        wqka = np.concatenate([u, cst])[None, :].astype(nbf)
        wv = wg[:, vcols].astype(nbf)
        uv = wg[:, vcols].sum(axis=0)[None, :].astype(nbf)
        wp = np.asarray(_CACHE["w_proj_rows"][g], nbf)
        in_maps.append({
            "xT": xT[b], "wqk": wqk, "wqka": wqka, "wv": wv, "uv": uv,
            "wp": wp, "masks": masks,
        })
    return in_maps


def prep_l2_inputs(hp_flat, ln2_g, ln2_b, w_fc, b_fc, w_fc2, b_fc2):
    g2 = np.asarray(ln2_g, np.float32)
    b2 = np.asarray(ln2_b, np.float32)
    wfc = np.asarray(w_fc, np.float32)
    bfc = np.asarray(b_fc, np.float32)
    w2 = np.asarray(w_fc2, np.float32)
    b22 = np.asarray(b_fc2, np.float32)

    wfc_g = (wfc * g2[:, None]).astype(nbf)
    wfc_t = np.ascontiguousarray(
        wfc_g.reshape(8, 128, 32, 128).transpose(2, 0, 1, 3))  # [32ct, 8ci, 128, 128]
    cfc = (b2 @ wfc + bfc).astype(np.float32).reshape(32, 128).T.copy()  # [128, 32]
    w2_t = np.ascontiguousarray(
        w2.astype(nbf).reshape(32, 128, 8, 128).transpose(2, 0, 1, 3))  # [8ct2, 32ci,...]
    b2cols = b22.astype(np.float32).reshape(8, 128).T.copy()

    in_maps = []
    for c in range(NCORES):
        rows = slice(c * ROWS2, (c + 1) * ROWS2)
        hT = np.ascontiguousarray(hp_flat[rows].T).astype(nbf)
        in_maps.append({
            "hT": hT, "wfc": wfc_t, "cfc": cfc,
            "w2": w2_t, "b2": b2cols,
        })
    return in_maps


def combine_l1(hidden_states, parts, b_attn, ln1_b, w_attn, b_proj):
    h = np.asarray(hidden_states, np.float32)
    wa = np.asarray(w_attn, np.float32)
    const_v = (np.asarray(ln1_b, np.float32) @ wa[:, 2 * D:]
               + np.asarray(b_attn, np.float32)[2 * D:])
    y_const = const_v @ np.asarray(_CACHE["w_proj_full"], np.float32) \
        + np.asarray(b_proj, np.float32)
    hp = h.copy()
    for b in range(B):
        acc = np.zeros((S, D), np.float32)
        for g in range(HG):
            acc += parts[b * HG + g]
        hp[b] += acc + y_const[None, :]
    return hp


def kernel(hidden_states, ln1_g, ln1_b, w_attn, b_attn, w_proj, b_proj,
           ln2_g, ln2_b, w_fc, b_fc, w_fc2, b_fc2):
    wpj = np.asarray(w_proj, np.float32)
    _CACHE["w_proj_full"] = wpj
    _CACHE["w_proj_rows"] = [
        np.concatenate([wpj[DH * (HG * g + i):DH * (HG * g + i) + DH, :]
                        for i in range(HG)], axis=0)
        for g in range(HG)
    ]

    nc1 = _get_l1()
    in1 = prep_l1_inputs(hidden_states, ln1_g, ln1_b, w_attn, b_attn)
    res1 = run_bass_kernel_spmd(nc1, in1, list(range(NCORES)))
    parts = [res1.results[c]["out"] for c in range(NCORES)]

    hp = combine_l1(hidden_states, parts, b_attn, ln1_b, w_attn, b_proj)
    hp_flat = hp.reshape(B * S, D)

    nc2 = _get_l2()
    in2 = prep_l2_inputs(hp_flat, ln2_g, ln2_b, w_fc, b_fc, w_fc2, b_fc2)
    res2 = run_bass_kernel_spmd(nc2, in2, list(range(NCORES)))

    out = hp_flat.copy()
    for c in range(NCORES):
        out[c * ROWS2:(c + 1) * ROWS2] += res2.results[c]["out"].T
    return out.reshape(B, S, D).astype(np.float32)

